# revision 6
# baseline (speedup 1.0000x reference)
"""NonLocal2D (attention) block on 8 trn2 NeuronCores — fp8 edition.

Sharding: core c -> batch n = c//2, query-half qh = c%2 (2048 of the 4096
spatial positions). Each core gets full x[n] (fp8, key-permuted so the query
window is always sbuf cols 0:2048), plus a bf16 residual slice; produces
out[n][:, qh*2048:(qh+1)*2048] in f32.

Per-core math (weight prep on host):
  A    = (w_theta^T @ w_phi) * 1024            e4m3 [256, 256]
  u    = A^T x[:, qwin]                        [256, Q]  PE DoubleRow -> e4m3
  g    = (w_g*32) @ x                          [128, N]  PE DoubleRow -> e4m3
  s    = x^T u  per s-tile: out[s, q] = x_q^T A x_s = 1024*theta(q).phi(s)
  B    = exp(s*sqrt(128)/1024 - M0)            -> e5m2  (ACT; every 3rd tile
         via DVE int16-Schraudolph + gpsimd clamp -> uint8 bitcast)
  y   += gpair^T B     (DoubleRow e4m3 x e5m2, PSUM accum over 16 pairs)
  d   += ones^T B      (DoubleRow e5m2, PSUM accum)
  ynt  = y * approx(1/d) -> bf16 ; r = (w_out/32)^T ynt ; out = x_bf16 + r

The key (s) order is permuted per-core on host; softmax/y sum over s, so any
order works as long as g-tiles and score-tiles agree.

Queries run in 2 passes of 1024 so PSUM fits:
  scores 2x[128,1024] + y [128,1024] + d [128,1024] = 8 banks = 16KB/part.
"""

import numpy as np
import ml_dtypes

import concourse.bass as bass
import concourse.mybir as mybir
import concourse.tile as tile
from concourse import bacc
from concourse.bass import ts
from concourse.bass_utils import run_bass_kernel_spmd

BF16 = mybir.dt.bfloat16
F32 = mybir.dt.float32
E4 = mybir.dt.float8e4
E5 = mybir.dt.float8e5
I16 = mybir.dt.int16
U8 = mybir.dt.uint8
AF = mybir.ActivationFunctionType
ALU = mybir.AluOpType
DR = mybir.MatmulPerfMode.DoubleRow

C = 256
CI = 128
NB = 4
N = 4096
Q = 2048          # queries per core
QP = 1024         # queries per pass
NCORES = 8

SCALE = float(128 ** 0.5)
ASC = 1024.0      # A prescale
GSC = 32.0        # g prescale
M0 = 15.0         # exp shift: B = exp(s_scaled - M0)
SC_ACT = SCALE / ASC
A8C = 4.0 / float(np.log(2.0))      # e5m2 bits per unit exponent
S_BITS1 = SC_ACT * A8C
S_BITS2 = 60.0 - A8C * M0

_CACHE: dict = {}


def _build():
    nc = bacc.Bacc("TRN2", target_bir_lowering=False, debug=False)

    d = {}
    d["xb8"] = nc.dram_tensor("xb8", [2, 4, 128, 1024], E4,
                              kind="ExternalInput").ap()
    d["xq16"] = nc.dram_tensor("xq16", [2, 128, Q], BF16,
                               kind="ExternalInput").ap()
    d["A8"] = nc.dram_tensor("A8", [2, 128, 2, 128], E4,
                             kind="ExternalInput").ap()
    d["wg8"] = nc.dram_tensor("wg8", [128, 2, 128], E4,
                              kind="ExternalInput").ap()
    d["wo16"] = nc.dram_tensor("wo16", [128, C], BF16,
                               kind="ExternalInput").ap()
    d["out"] = nc.dram_tensor("out", [2, 128, Q], F32, kind="ExternalOutput").ap()

    with tile.TileContext(nc) as tc:
        _body(tc, d)
    nc.compile()
    return nc


def _body(tc, d):
    nc = tc.nc

    with (
        tc.tile_pool(name="const", bufs=1) as const,
        tc.tile_pool(name="acts", bufs=1) as acts,
        tc.tile_pool(name="bp", bufs=1) as bp,
        tc.tile_pool(name="bip", bufs=3) as bip,
        tc.tile_pool(name="outs", bufs=2) as outp,
        tc.tile_pool(name="scp", bufs=2, space="PSUM") as scp,
        tc.tile_pool(name="yd", bufs=1, space="PSUM") as yd,
    ):
        # ---- tiny constants on DVE (keeps gpsimd free for DMA triggers) ----
        wup_l = const.tile([128, 128], BF16, tag="wup_l")
        wup_r = const.tile([128, 256], BF16, tag="wup_r")
        nc.vector.memset(wup_l[:], 1.0)
        nc.vector.memset(wup_r[:], 0.0)
        scratch = const.tile([128, 1], BF16, tag="scratch")
        nc.scalar.activation(scratch[:], wup_l[:, 0:1], AF.Exp, scale=1.0)

        A_sb = const.tile([128, 2, 2, 128], E4, tag="A")     # [kp, h, j, m]
        wg_sb = const.tile([128, 2, 128], E4, tag="wg")
        wo_sb = const.tile([128, C], BF16, tag="wo")
        xb_sb = acts.tile([128, 2, N], E4, tag="xb")         # [kp, kc, s]
        xq_sb = acts.tile([128, 2, Q], BF16, tag="xq")

        # fill: sync gets A8 + kc0 quarters + xq; gpsimd kc1 quarters + w
        nc.sync.dma_start(out=A_sb[:, 0], in_=d["A8"][0])
        nc.sync.dma_start(out=A_sb[:, 1], in_=d["A8"][1])
        nc.gpsimd.dma_start(out=wg_sb[:], in_=d["wg8"][:])
        for qt in range(4):
            nc.sync.dma_start(out=xb_sb[:, 0, ts(qt, 1024)], in_=d["xb8"][0][qt])
            nc.gpsimd.dma_start(out=xb_sb[:, 1, ts(qt, 1024)], in_=d["xb8"][1][qt])
            if qt == 0:
                m0t = const.tile([128, 1], F32, tag="m0")
                ones5 = const.tile([128, 2, 128], E5, tag="ones5")
                nc.gpsimd.memset(m0t[:], -M0)
                nc.gpsimd.memset(ones5[:], 1.0)
            if qt == 1:
                nc.gpsimd.dma_start(out=wo_sb[:], in_=d["wo16"][:])
        for oc in range(2):
            nc.sync.dma_start(out=xq_sb[:, oc], in_=d["xq16"][oc])

        u8_sb = acts.tile([128, 2, Q], E4, tag="u8")         # [kp, h, q]
        g5_sb = acts.tile([128, 16, 2, 128], E4, tag="g5")   # [sp, pair, slot, o]

        yps = yd.tile([128, QP], F32, tag="yps")
        dps = yd.tile([128, QP], F32, tag="dps")

        # ---- PE warm-up during the fill (clock ramp) ----
        for i in range(12):
            nc.tensor.matmul([yps, dps][i % 2][:, 0:256], wup_l[:], wup_r[:],
                             start=True, stop=True)

        def cast(eng, dst, src):
            if eng is nc.scalar:
                nc.scalar.copy(dst, src)
            else:
                eng.tensor_copy(dst, src)

        # ---- u = A^T x[:, 0:2048]: 4 independent psum tiles, casts ACT/DVE ----
        for r in range(2):
            for h in range(2):
                i = 2 * r + h
                pt = ([yps, dps][i] if i < 2 else
                      scp.tile([128, QP], F32, tag="sc", name=f"u{i}"))
                for ch in range(2):
                    nc.tensor.matmul(
                        pt[:, ts(ch, 512)],
                        A_sb[:, h],
                        xb_sb[:, :, r * QP + ch * 512:r * QP + (ch + 1) * 512],
                        start=True, stop=True, perf_mode=DR)
                cast([nc.scalar, nc.vector][h], u8_sb[:, h, ts(r, QP)], pt[:])

        # ---- g: 4 rounds x 8 s-tiles into scp tiles ----
        for b in range(4):
            gp = scp.tile([128, QP], F32, tag="sc", name=f"g{b}")
            for sj in range(8):
                st = b * 8 + sj
                nc.tensor.matmul(
                    gp[:, ts(sj, 128)],
                    xb_sb[:, :, ts(st, 128)],
                    wg_sb[:],
                    start=True, stop=True, perf_mode=DR)
            # s-tiles 8b..8b+7 = pairs 4b..4b+3
            cast([nc.scalar, nc.vector][b % 2],
                 g5_sb[:, 4 * b:4 * b + 4, :, :], gp[:])

        # ---- main: 2 passes x 32 s-tiles ----
        for p in range(2):
            qoff = p * QP
            Bt = {}
            for t in range(32):
                pair, slot = t // 2, t % 2
                if slot == 0:
                    Bt[pair] = bp.tile([128, 2, QP], E5, tag=f"B{pair}",
                                       name=f"B{p}_{pair}")
                sc = scp.tile([128, QP], F32, tag="sc", name=f"s{p}_{t}")
                for ch in range(2):
                    nc.tensor.matmul(
                        sc[:, ts(ch, 512)],
                        xb_sb[:, :, ts(t, 128)],
                        u8_sb[:, :, qoff + ch * 512:qoff + (ch + 1) * 512],
                        start=True, stop=True, perf_mode=DR)
                Bslot = Bt[pair][:, slot, :]
                if t % 3 == 2:
                    # offload: DVE Schraudolph bits -> gpsimd clamp/convert
                    bi = bip.tile([128, QP], I16, tag="bi", name=f"bi{p}_{t}")
                    nc.vector.tensor_scalar(
                        out=bi[:], in0=sc[:],
                        scalar1=S_BITS1, scalar2=S_BITS2,
                        op0=ALU.mult, op1=ALU.add)
                    nc.gpsimd.tensor_scalar(
                        out=Bslot.bitcast(U8), in0=bi[:],
                        scalar1=0.0, scalar2=0.0,
                        op0=ALU.max, op1=ALU.add)
                else:
                    nc.scalar.activation(Bslot, sc[:], AF.Exp,
                                         bias=m0t[:], scale=SC_ACT)
                if slot == 1:
                    for ch in range(2):
                        nc.tensor.matmul(
                            yps[:, ts(ch, 512)],
                            g5_sb[:, pair],
                            Bt[pair][:, :, ts(ch, 512)],
                            start=(pair == 0), stop=(pair == 15),
                            perf_mode=DR)
                    for ch in range(2):
                        nc.tensor.matmul(
                            dps[:, ts(ch, 512)],
                            ones5[:],
                            Bt[pair][:, :, ts(ch, 512)],
                            start=(pair == 0), stop=(pair == 15),
                            perf_mode=DR)

            # ---- pass tail ----
            rcp = outp.tile([128, QP], F32, tag="rcp", name=f"rcp{p}")
            nc.vector.reciprocal_approx_fast(rcp[:], dps[:])
            ynt = outp.tile([128, QP], BF16, tag="ynt", name=f"ynt{p}")
            nc.vector.tensor_tensor(ynt[:], yps[:], rcp[:], ALU.mult)
            for oc in range(2):
                rp = scp.tile([128, QP], F32, tag="sc", name=f"rp{p}_{oc}")
                for ch in range(2):
                    nc.tensor.matmul(
                        rp[:, ts(ch, 512)],
                        wo_sb[:, ts(oc, 128)],
                        ynt[:, ts(ch, 512)],
                        start=True, stop=True)
                ot = outp.tile([128, QP], F32, tag=f"ot{oc}", name=f"ot{p}_{oc}")
                nc.vector.tensor_tensor(
                    ot[:], rp[:], xq_sb[:, oc, ts(p, QP)], ALU.add)
                [nc.sync, nc.gpsimd][oc].dma_start(
                    out=d["out"][oc][:, ts(p, QP)], in_=ot[:])


def _prep_in_maps(inputs):
    bf = ml_dtypes.bfloat16
    e4 = ml_dtypes.float8_e4m3
    x = np.ascontiguousarray(np.asarray(inputs["x"], dtype=np.float32))
    w_g = np.asarray(inputs["w_g"], np.float32)
    w_theta = np.asarray(inputs["w_theta"], np.float32)
    w_phi = np.asarray(inputs["w_phi"], np.float32)
    w_out = np.asarray(inputs["w_out"], np.float32)
    for bn in ("b_g", "b_theta", "b_phi", "b_out"):
        assert not np.any(np.asarray(inputs[bn])), f"nonzero {bn} unsupported"

    A = (w_theta.T @ w_phi) * ASC                      # [256, 256]
    # u-projection lhsT: A8[h][kp, j, m] = A[j*128+kp, h*128+m]
    A8 = np.empty((2, 128, 2, 128), e4)
    for h in range(2):
        for j in range(2):
            A8[h, :, j, :] = A[j * 128:(j + 1) * 128,
                               h * 128:(h + 1) * 128].astype(e4)
    wg32 = w_g * GSC                                   # [128, 256]
    wg8 = np.empty((128, 2, 128), e4)
    for kc in range(2):
        wg8[:, kc, :] = wg32[:, kc * 128:(kc + 1) * 128].T.astype(e4)
    wo16 = np.ascontiguousarray((w_out / GSC).T).astype(bf)   # [CI, C]

    in_maps = []
    for c in range(NCORES):
        n, qh = c // 2, c % 2
        xr = x[n].reshape(C, N)
        x8 = xr.astype(e4)
        # key-permute: query window first
        if qh == 1:
            x8 = np.concatenate([x8[:, Q:], x8[:, :Q]], axis=1)
        m = {
            "xb8": np.ascontiguousarray(
                x8.reshape(2, 128, 4, 1024).transpose(0, 2, 1, 3)),
            "xq16": np.ascontiguousarray(
                xr[:, qh * Q:(qh + 1) * Q].astype(bf).reshape(2, 128, Q)),
            "A8": A8, "wg8": wg8, "wo16": wo16,
        }
        in_maps.append(m)
    return in_maps


def _get_nc():
    if "nc" not in _CACHE:
        _CACHE["nc"] = _build()
    return _CACHE["nc"]


def kernel(**inputs):
    in_maps = _prep_in_maps(inputs)
    nc = _get_nc()
    res = run_bass_kernel_spmd(nc, in_maps, list(range(NCORES)))
    out = np.empty((NB, C, N), np.float32)
    for c in range(NCORES):
        n, qh = c // 2, c % 2
        out[n][:, qh * Q:(qh + 1) * Q] = res.results[c]["out"].reshape(C, Q)
    return out.reshape(NB, C, 64, 64)


if __name__ == "__main__":
    rng = np.random.default_rng(0)
    ins = {
        "x": rng.normal(size=(NB, C, 64, 64)).astype(np.float32),
        "w_g": rng.normal(size=(CI, C)).astype(np.float32) * 0.01,
        "b_g": np.zeros(CI, np.float32),
        "w_theta": rng.normal(size=(CI, C)).astype(np.float32) * 0.01,
        "b_theta": np.zeros(CI, np.float32),
        "w_phi": rng.normal(size=(CI, C)).astype(np.float32) * 0.01,
        "b_phi": np.zeros(CI, np.float32),
        "w_out": rng.normal(size=(C, CI)).astype(np.float32) * 0.01,
        "b_out": np.zeros(C, np.float32),
    }
    o = kernel(**ins)
    print("ok", o.shape, o.dtype)


# revision 7
# speedup vs baseline: 3.1200x; 3.1200x over previous
"""NonLocal2D (attention) block on 8 trn2 NeuronCores — fp8 edition.

Sharding: core c -> batch n = c//2, query-half qh = c%2 (2048 of the 4096
spatial positions). Each core gets full x[n] (fp8, key-permuted so the query
window is always sbuf cols 0:2048), plus a bf16 residual slice; produces
out[n][:, qh*2048:(qh+1)*2048] in f32.

Per-core math (weight prep on host):
  A    = (w_theta^T @ w_phi) * 1024            e4m3 [256, 256]
  u    = A^T x[:, qwin]                        [256, Q]  PE DoubleRow -> e4m3
  g    = (w_g*32) @ x                          [128, N]  PE DoubleRow -> e4m3
  s    = x^T u  per s-tile: out[s, q] = x_q^T A x_s = 1024*theta(q).phi(s)
  B    = exp(s*sqrt(128)/1024 - M0)            -> e5m2  (ACT; every 3rd tile
         via DVE int16-Schraudolph + gpsimd clamp -> uint8 bitcast)
  y   += gpair^T B     (DoubleRow e4m3 x e5m2, PSUM accum over 16 pairs)
  d   += ones^T B      (DoubleRow e5m2, PSUM accum)
  ynt  = y * approx(1/d) -> bf16 ; r = (w_out/32)^T ynt ; out = x_bf16 + r

The key (s) order is permuted per-core on host; softmax/y sum over s, so any
order works as long as g-tiles and score-tiles agree.

Queries run in 2 passes of 1024 so PSUM fits:
  scores 2x[128,1024] + y [128,1024] + d [128,1024] = 8 banks = 16KB/part.
"""

import numpy as np
import ml_dtypes

import concourse.bass as bass
import concourse.mybir as mybir
import concourse.tile as tile
from concourse import bacc
from concourse.bass import ts
from concourse.bass_utils import run_bass_kernel_spmd

BF16 = mybir.dt.bfloat16
F32 = mybir.dt.float32
E4 = mybir.dt.float8e4
E5 = mybir.dt.float8e5
I16 = mybir.dt.int16
U8 = mybir.dt.uint8
AF = mybir.ActivationFunctionType
ALU = mybir.AluOpType
DR = mybir.MatmulPerfMode.DoubleRow

C = 256
CI = 128
NB = 4
N = 4096
Q = 2048          # queries per core
QP = 1024         # queries per pass
NCORES = 8

SCALE = float(128 ** 0.5)
ASC = 1024.0      # A prescale
GSC = 32.0        # g prescale
M0 = 15.0         # exp shift: B = exp(s_scaled - M0)
SC_ACT = SCALE / ASC
A8C = 4.0 / float(np.log(2.0))      # e5m2 bits per unit exponent
S_BITS1 = SC_ACT * A8C
S_BITS2 = 60.0 - A8C * M0

_CACHE: dict = {}


def _build():
    nc = bacc.Bacc("TRN2", target_bir_lowering=False, debug=False)

    d = {}
    d["xb8"] = nc.dram_tensor("xb8", [2, 4, 128, 1024], E4,
                              kind="ExternalInput").ap()
    d["xq16"] = nc.dram_tensor("xq16", [2, 128, Q], BF16,
                               kind="ExternalInput").ap()
    d["A8"] = nc.dram_tensor("A8", [2, 128, 2, 128], E4,
                             kind="ExternalInput").ap()
    d["wg8"] = nc.dram_tensor("wg8", [128, 2, 128], E4,
                              kind="ExternalInput").ap()
    d["wo16"] = nc.dram_tensor("wo16", [128, C], BF16,
                               kind="ExternalInput").ap()
    d["out"] = nc.dram_tensor("out", [2, 128, Q], F32, kind="ExternalOutput").ap()

    with tile.TileContext(nc) as tc:
        _body(tc, d)
    nc.compile()
    return nc


def _body(tc, d):
    nc = tc.nc

    with (
        tc.tile_pool(name="const", bufs=1) as const,
        tc.tile_pool(name="acts", bufs=1) as acts,
        tc.tile_pool(name="bp", bufs=1) as bp,
        tc.tile_pool(name="bip", bufs=3) as bip,
        tc.tile_pool(name="outs", bufs=2) as outp,
        tc.tile_pool(name="scp", bufs=2, space="PSUM") as scp,
        tc.tile_pool(name="yd", bufs=1, space="PSUM") as yd,
    ):
        # ---- tiny constants on DVE (keeps gpsimd free for DMA triggers) ----
        wup_l = const.tile([128, 128], BF16, tag="wup_l")
        wup_r = const.tile([128, 256], BF16, tag="wup_r")
        nc.vector.memset(wup_l[:], 1.0)
        nc.vector.memset(wup_r[:], 0.0)
        scratch = const.tile([128, 1], BF16, tag="scratch")
        nc.scalar.activation(scratch[:], wup_l[:, 0:1], AF.Exp, scale=1.0)

        A_sb = const.tile([128, 2, 2, 128], E4, tag="A")     # [kp, h, j, m]
        wg_sb = const.tile([128, 2, 128], E4, tag="wg")
        wo_sb = const.tile([128, C], BF16, tag="wo")
        xb_sb = acts.tile([128, 2, N], E4, tag="xb")         # [kp, kc, s]
        xq_sb = acts.tile([128, 2, Q], BF16, tag="xq")

        # fill: sync gets A8 + kc0 quarters + xq; gpsimd kc1 quarters + w
        nc.sync.dma_start(out=A_sb[:, 0], in_=d["A8"][0])
        nc.sync.dma_start(out=A_sb[:, 1], in_=d["A8"][1])
        nc.gpsimd.dma_start(out=wg_sb[:], in_=d["wg8"][:])
        for qt in range(4):
            nc.sync.dma_start(out=xb_sb[:, 0, ts(qt, 1024)], in_=d["xb8"][0][qt])
            nc.gpsimd.dma_start(out=xb_sb[:, 1, ts(qt, 1024)], in_=d["xb8"][1][qt])
            if qt == 0:
                m0t = const.tile([128, 1], F32, tag="m0")
                ones5 = const.tile([128, 2, 128], E5, tag="ones5")
                nc.gpsimd.memset(m0t[:], -M0)
                nc.gpsimd.memset(ones5[:], 1.0)
            if qt == 1:
                nc.gpsimd.dma_start(out=wo_sb[:], in_=d["wo16"][:])
        for oc in range(2):
            nc.sync.dma_start(out=xq_sb[:, oc], in_=d["xq16"][oc])

        u8_sb = acts.tile([128, 2, Q], E4, tag="u8")         # [kp, h, q]
        g5_sb = acts.tile([128, 16, 2, 128], E4, tag="g5")   # [sp, pair, slot, o]

        yps = yd.tile([128, QP], F32, tag="yps")
        dps = yd.tile([128, QP], F32, tag="dps")

        # ---- PE warm-up during the fill (clock ramp) ----
        for i in range(12):
            nc.tensor.matmul([yps, dps][i % 2][:, 0:256], wup_l[:], wup_r[:],
                             start=True, stop=True)

        def cast(eng, dst, src):
            if eng is nc.scalar:
                nc.scalar.copy(dst, src)
            else:
                eng.tensor_copy(dst, src)

        # ---- u = A^T x[:, 0:2048]: 4 independent psum tiles, casts ACT/DVE ----
        for r in range(2):
            for h in range(2):
                i = 2 * r + h
                pt = ([yps, dps][i] if i < 2 else
                      scp.tile([128, QP], F32, tag="sc", name=f"u{i}"))
                for ch in range(2):
                    nc.tensor.matmul(
                        pt[:, ts(ch, 512)],
                        A_sb[:, h],
                        xb_sb[:, :, r * QP + ch * 512:r * QP + (ch + 1) * 512],
                        start=True, stop=True, perf_mode=DR)
                cast([nc.scalar, nc.vector][h], u8_sb[:, h, ts(r, QP)], pt[:])

        # ---- g: 4 rounds x 8 s-tiles into scp tiles ----
        for b in range(4):
            gp = scp.tile([128, QP], F32, tag="sc", name=f"g{b}")
            for sj in range(8):
                st = b * 8 + sj
                nc.tensor.matmul(
                    gp[:, ts(sj, 128)],
                    xb_sb[:, :, ts(st, 128)],
                    wg_sb[:],
                    start=True, stop=True, perf_mode=DR)
            # s-tiles 8b..8b+7 = pairs 4b..4b+3
            cast([nc.scalar, nc.vector][b % 2],
                 g5_sb[:, 4 * b:4 * b + 4, :, :], gp[:])

        # ---- main: 2 passes x 32 s-tiles ----
        for p in range(2):
            qoff = p * QP
            Bt = {}
            for t in range(32):
                pair, slot = t // 2, t % 2
                if slot == 0:
                    Bt[pair] = bp.tile([128, 2, QP], E5, tag=f"B{pair}",
                                       name=f"B{p}_{pair}")
                sc = scp.tile([128, QP], F32, tag="sc", name=f"s{p}_{t}")
                for ch in range(2):
                    nc.tensor.matmul(
                        sc[:, ts(ch, 512)],
                        xb_sb[:, :, ts(t, 128)],
                        u8_sb[:, :, qoff + ch * 512:qoff + (ch + 1) * 512],
                        start=True, stop=True, perf_mode=DR)
                Bslot = Bt[pair][:, slot, :]
                if t % 4 == 2:
                    # offload: DVE Schraudolph bits, then DVE clamp/convert
                    # (gpsimd tensor ops are ~15us each on HW - unusable)
                    bi = bip.tile([128, QP], I16, tag="bi", name=f"bi{p}_{t}")
                    nc.vector.tensor_scalar(
                        out=bi[:], in0=sc[:],
                        scalar1=S_BITS1, scalar2=S_BITS2,
                        op0=ALU.mult, op1=ALU.add)
                    nc.vector.tensor_scalar(
                        out=Bslot.bitcast(U8), in0=bi[:],
                        scalar1=0.0, scalar2=0.0,
                        op0=ALU.max, op1=ALU.add)
                else:
                    nc.scalar.activation(Bslot, sc[:], AF.Exp,
                                         bias=m0t[:], scale=SC_ACT)
                if slot == 1:
                    for ch in range(2):
                        nc.tensor.matmul(
                            yps[:, ts(ch, 512)],
                            g5_sb[:, pair],
                            Bt[pair][:, :, ts(ch, 512)],
                            start=(pair == 0), stop=(pair == 15),
                            perf_mode=DR)
                    for ch in range(2):
                        nc.tensor.matmul(
                            dps[:, ts(ch, 512)],
                            ones5[:],
                            Bt[pair][:, :, ts(ch, 512)],
                            start=(pair == 0), stop=(pair == 15),
                            perf_mode=DR)

            # ---- pass tail ----
            rcp = outp.tile([128, QP], F32, tag="rcp", name=f"rcp{p}")
            nc.vector.reciprocal_approx_fast(rcp[:], dps[:])
            ynt = outp.tile([128, QP], BF16, tag="ynt", name=f"ynt{p}")
            nc.vector.tensor_tensor(ynt[:], yps[:], rcp[:], ALU.mult)
            for oc in range(2):
                rp = scp.tile([128, QP], F32, tag="sc", name=f"rp{p}_{oc}")
                for ch in range(2):
                    nc.tensor.matmul(
                        rp[:, ts(ch, 512)],
                        wo_sb[:, ts(oc, 128)],
                        ynt[:, ts(ch, 512)],
                        start=True, stop=True)
                ot = outp.tile([128, QP], F32, tag=f"ot{oc}", name=f"ot{p}_{oc}")
                nc.vector.tensor_tensor(
                    ot[:], rp[:], xq_sb[:, oc, ts(p, QP)], ALU.add)
                [nc.sync, nc.gpsimd][oc].dma_start(
                    out=d["out"][oc][:, ts(p, QP)], in_=ot[:])


def _prep_in_maps(inputs):
    bf = ml_dtypes.bfloat16
    e4 = ml_dtypes.float8_e4m3
    x = np.ascontiguousarray(np.asarray(inputs["x"], dtype=np.float32))
    w_g = np.asarray(inputs["w_g"], np.float32)
    w_theta = np.asarray(inputs["w_theta"], np.float32)
    w_phi = np.asarray(inputs["w_phi"], np.float32)
    w_out = np.asarray(inputs["w_out"], np.float32)
    for bn in ("b_g", "b_theta", "b_phi", "b_out"):
        assert not np.any(np.asarray(inputs[bn])), f"nonzero {bn} unsupported"

    A = (w_theta.T @ w_phi) * ASC                      # [256, 256]
    # u-projection lhsT: A8[h][kp, j, m] = A[j*128+kp, h*128+m]
    A8 = np.empty((2, 128, 2, 128), e4)
    for h in range(2):
        for j in range(2):
            A8[h, :, j, :] = A[j * 128:(j + 1) * 128,
                               h * 128:(h + 1) * 128].astype(e4)
    wg32 = w_g * GSC                                   # [128, 256]
    wg8 = np.empty((128, 2, 128), e4)
    for kc in range(2):
        wg8[:, kc, :] = wg32[:, kc * 128:(kc + 1) * 128].T.astype(e4)
    wo16 = np.ascontiguousarray((w_out / GSC).T).astype(bf)   # [CI, C]

    in_maps = []
    for c in range(NCORES):
        n, qh = c // 2, c % 2
        xr = x[n].reshape(C, N)
        x8 = xr.astype(e4)
        # key-permute: query window first
        if qh == 1:
            x8 = np.concatenate([x8[:, Q:], x8[:, :Q]], axis=1)
        m = {
            "xb8": np.ascontiguousarray(
                x8.reshape(2, 128, 4, 1024).transpose(0, 2, 1, 3)),
            "xq16": np.ascontiguousarray(
                xr[:, qh * Q:(qh + 1) * Q].astype(bf).reshape(2, 128, Q)),
            "A8": A8, "wg8": wg8, "wo16": wo16,
        }
        in_maps.append(m)
    return in_maps


def _get_nc():
    if "nc" not in _CACHE:
        _CACHE["nc"] = _build()
    return _CACHE["nc"]


def kernel(**inputs):
    in_maps = _prep_in_maps(inputs)
    nc = _get_nc()
    res = run_bass_kernel_spmd(nc, in_maps, list(range(NCORES)))
    out = np.empty((NB, C, N), np.float32)
    for c in range(NCORES):
        n, qh = c // 2, c % 2
        out[n][:, qh * Q:(qh + 1) * Q] = res.results[c]["out"].reshape(C, Q)
    return out.reshape(NB, C, 64, 64)


if __name__ == "__main__":
    rng = np.random.default_rng(0)
    ins = {
        "x": rng.normal(size=(NB, C, 64, 64)).astype(np.float32),
        "w_g": rng.normal(size=(CI, C)).astype(np.float32) * 0.01,
        "b_g": np.zeros(CI, np.float32),
        "w_theta": rng.normal(size=(CI, C)).astype(np.float32) * 0.01,
        "b_theta": np.zeros(CI, np.float32),
        "w_phi": rng.normal(size=(CI, C)).astype(np.float32) * 0.01,
        "b_phi": np.zeros(CI, np.float32),
        "w_out": rng.normal(size=(C, CI)).astype(np.float32) * 0.01,
        "b_out": np.zeros(C, np.float32),
    }
    o = kernel(**ins)
    print("ok", o.shape, o.dtype)


# revision 8
# speedup vs baseline: 3.5219x; 1.1288x over previous
"""NonLocal2D (attention) block on 8 trn2 NeuronCores — fp8 edition.

Sharding: core c -> batch n = c//2, query-half qh = c%2 (2048 of the 4096
spatial positions). Each core gets full x[n] (fp8, key-permuted so the query
window is always sbuf cols 0:2048), plus a bf16 residual slice; produces
out[n][:, qh*2048:(qh+1)*2048] in f32.

Per-core math (weight prep on host):
  A    = (w_theta^T @ w_phi) * 1024            e4m3 [256, 256]
  u    = A^T x[:, qwin]                        [256, Q]  PE DoubleRow -> e4m3
  g    = (w_g*32) @ x                          [128, N]  PE DoubleRow -> e4m3
  s    = x^T u  per s-tile: out[s, q] = x_q^T A x_s = 1024*theta(q).phi(s)
  B    = exp(s*sqrt(128)/1024 - M0)            -> e5m2  (ACT; every 3rd tile
         via DVE int16-Schraudolph + gpsimd clamp -> uint8 bitcast)
  y   += gpair^T B     (DoubleRow e4m3 x e5m2, PSUM accum over 16 pairs)
  d   += ones^T B      (DoubleRow e5m2, PSUM accum)
  ynt  = y * approx(1/d) -> bf16 ; r = (w_out/32)^T ynt ; out = x_bf16 + r

The key (s) order is permuted per-core on host; softmax/y sum over s, so any
order works as long as g-tiles and score-tiles agree.

Queries run in 2 passes of 1024 so PSUM fits:
  scores 2x[128,1024] + y [128,1024] + d [128,1024] = 8 banks = 16KB/part.
"""

import numpy as np
import ml_dtypes

import concourse.bass as bass
import concourse.mybir as mybir
import concourse.tile as tile
from concourse import bacc
from concourse.bass import ts
from concourse.bass_utils import run_bass_kernel_spmd

BF16 = mybir.dt.bfloat16
F32 = mybir.dt.float32
E4 = mybir.dt.float8e4
E5 = mybir.dt.float8e5
I16 = mybir.dt.int16
U8 = mybir.dt.uint8
AF = mybir.ActivationFunctionType
ALU = mybir.AluOpType
DR = mybir.MatmulPerfMode.DoubleRow

C = 256
CI = 128
NB = 4
N = 4096
Q = 2048          # queries per core
QP = 1024         # queries per pass
NCORES = 8

SCALE = float(128 ** 0.5)
ASC = 1024.0      # A prescale
GSC = 32.0        # g prescale
M0 = 15.0         # exp shift: B = exp(s_scaled - M0)
SC_ACT = SCALE / ASC
A8C = 4.0 / float(np.log(2.0))      # e5m2 bits per unit exponent
S_BITS1 = SC_ACT * A8C
S_BITS2 = 60.0 - A8C * M0

_CACHE: dict = {}


def _build():
    nc = bacc.Bacc("TRN2", target_bir_lowering=False, debug=False)

    d = {}
    d["xb8"] = nc.dram_tensor("xb8", [2, 4, 128, 1024], E4,
                              kind="ExternalInput").ap()
    d["xq16"] = nc.dram_tensor("xq16", [2, 128, Q], BF16,
                               kind="ExternalInput").ap()
    d["A8"] = nc.dram_tensor("A8", [2, 128, 2, 128], E4,
                             kind="ExternalInput").ap()
    d["wg8"] = nc.dram_tensor("wg8", [128, 2, 128], E4,
                              kind="ExternalInput").ap()
    d["wo16"] = nc.dram_tensor("wo16", [128, C], BF16,
                               kind="ExternalInput").ap()
    d["out"] = nc.dram_tensor("out", [2, 128, Q], F32, kind="ExternalOutput").ap()

    with tile.TileContext(nc) as tc:
        _body(tc, d)
    nc.compile()
    return nc


def _body(tc, d):
    nc = tc.nc

    with (
        tc.tile_pool(name="const", bufs=1) as const,
        tc.tile_pool(name="acts", bufs=1) as acts,
        tc.tile_pool(name="bp", bufs=1) as bp,
        tc.tile_pool(name="bip", bufs=3) as bip,
        tc.tile_pool(name="outs", bufs=2) as outp,
        tc.tile_pool(name="scp", bufs=2, space="PSUM") as scp,
        tc.tile_pool(name="yd", bufs=1, space="PSUM") as yd,
    ):
        # ---- tiny constants on DVE (keeps gpsimd free for DMA triggers) ----
        wup_l = const.tile([128, 128], BF16, tag="wup_l")
        wup_r = const.tile([128, 256], BF16, tag="wup_r")
        nc.vector.memset(wup_l[:], 1.0)
        nc.vector.memset(wup_r[:], 0.0)
        scratch = const.tile([128, 1], BF16, tag="scratch")
        nc.scalar.activation(scratch[:], wup_l[:, 0:1], AF.Exp, scale=1.0)

        A_sb = const.tile([128, 2, 2, 128], E4, tag="A")     # [kp, h, j, m]
        wg_sb = const.tile([128, 2, 128], E4, tag="wg")
        wo_sb = const.tile([128, C], BF16, tag="wo")
        xb_sb = acts.tile([128, 2, N], E4, tag="xb")         # [kp, kc, s]
        xq_sb = acts.tile([128, 2, Q], BF16, tag="xq")

        # fill: sync gets A8 + kc0 quarters + xq; gpsimd kc1 quarters + w
        nc.sync.dma_start(out=A_sb[:, 0], in_=d["A8"][0])
        nc.sync.dma_start(out=A_sb[:, 1], in_=d["A8"][1])
        nc.gpsimd.dma_start(out=wg_sb[:], in_=d["wg8"][:])
        for qt in range(4):
            nc.sync.dma_start(out=xb_sb[:, 0, ts(qt, 1024)], in_=d["xb8"][0][qt])
            nc.gpsimd.dma_start(out=xb_sb[:, 1, ts(qt, 1024)], in_=d["xb8"][1][qt])
            if qt == 0:
                m0t = const.tile([128, 1], F32, tag="m0")
                ones5 = const.tile([128, 2, 128], E5, tag="ones5")
                nc.gpsimd.memset(m0t[:], -M0)
                nc.gpsimd.memset(ones5[:], 1.0)
            if qt == 1:
                nc.gpsimd.dma_start(out=wo_sb[:], in_=d["wo16"][:])
        for oc in range(2):
            nc.sync.dma_start(out=xq_sb[:, oc], in_=d["xq16"][oc])

        u8_sb = acts.tile([128, 2, Q], E4, tag="u8")         # [kp, h, q]
        g5_sb = acts.tile([128, 16, 2, 128], E4, tag="g5")   # [sp, pair, slot, o]

        yps = yd.tile([128, QP], F32, tag="yps")
        dps = yd.tile([128, QP], F32, tag="dps")

        # ---- PE warm-up during the fill (clock ramp) ----
        for i in range(12):
            nc.tensor.matmul([yps, dps][i % 2][:, 0:256], wup_l[:], wup_r[:],
                             start=True, stop=True)

        def cast(eng, dst, src):
            if eng is nc.scalar:
                nc.scalar.copy(dst, src)
            else:
                eng.tensor_copy(dst, src)

        # ---- u = A^T x[:, 0:2048]: 4 independent psum tiles, casts ACT/DVE ----
        for r in range(2):
            for h in range(2):
                i = 2 * r + h
                pt = ([yps, dps][i] if i < 2 else
                      scp.tile([128, QP], F32, tag="sc", name=f"u{i}"))
                for ch in range(2):
                    nc.tensor.matmul(
                        pt[:, ts(ch, 512)],
                        A_sb[:, h],
                        xb_sb[:, :, r * QP + ch * 512:r * QP + (ch + 1) * 512],
                        start=True, stop=True, perf_mode=DR)
                cast([nc.scalar, nc.vector][h], u8_sb[:, h, ts(r, QP)], pt[:])

        # ---- g: 4 rounds x 8 s-tiles into scp tiles ----
        for b in range(4):
            gp = scp.tile([128, QP], F32, tag="sc", name=f"g{b}")
            for sj in range(8):
                st = b * 8 + sj
                nc.tensor.matmul(
                    gp[:, ts(sj, 128)],
                    xb_sb[:, :, ts(st, 128)],
                    wg_sb[:],
                    start=True, stop=True, perf_mode=DR)
            # s-tiles 8b..8b+7 = pairs 4b..4b+3
            cast([nc.scalar, nc.vector][b % 2],
                 g5_sb[:, 4 * b:4 * b + 4, :, :], gp[:])

        # ---- main: 2 passes x 32 s-tiles.  y/d matmuls are emitted 4 tiles
        # late so PE's in-order queue never parks scores (and thus exp)
        # behind a y/d that waits on B. ----
        def emit_yd(p, pair, Bt):
            for ch in range(2):
                nc.tensor.matmul(
                    yps[:, ts(ch, 512)],
                    g5_sb[:, pair],
                    Bt[pair][:, :, ts(ch, 512)],
                    start=(pair == 0), stop=(pair == 15),
                    perf_mode=DR)
            for ch in range(2):
                nc.tensor.matmul(
                    dps[:, ts(ch, 512)],
                    ones5[:],
                    Bt[pair][:, :, ts(ch, 512)],
                    start=(pair == 0), stop=(pair == 15),
                    perf_mode=DR)

        LAG = 4
        for p in range(2):
            qoff = p * QP
            Bt = {}
            for t in range(32):
                pair, slot = t // 2, t % 2
                if slot == 0:
                    Bt[pair] = bp.tile([128, 2, QP], E5, tag=f"B{pair}",
                                       name=f"B{p}_{pair}")
                sc = scp.tile([128, QP], F32, tag="sc", name=f"s{p}_{t}")
                for ch in range(2):
                    nc.tensor.matmul(
                        sc[:, ts(ch, 512)],
                        xb_sb[:, :, ts(t, 128)],
                        u8_sb[:, :, qoff + ch * 512:qoff + (ch + 1) * 512],
                        start=True, stop=True, perf_mode=DR)
                Bslot = Bt[pair][:, slot, :]
                if t % 4 == 2:
                    # offload: DVE Schraudolph bits, then DVE clamp/convert
                    # (gpsimd tensor ops are ~15us each on HW - unusable)
                    bi = bip.tile([128, QP], I16, tag="bi", name=f"bi{p}_{t}")
                    nc.vector.tensor_scalar(
                        out=bi[:], in0=sc[:],
                        scalar1=S_BITS1, scalar2=S_BITS2,
                        op0=ALU.mult, op1=ALU.add)
                    nc.vector.tensor_scalar(
                        out=Bslot.bitcast(U8), in0=bi[:],
                        scalar1=0.0, scalar2=0.0,
                        op0=ALU.max, op1=ALU.add)
                else:
                    nc.scalar.activation(Bslot, sc[:], AF.Exp,
                                         bias=m0t[:], scale=SC_ACT)
                if t >= LAG + 1 and (t - LAG) % 2 == 1:
                    emit_yd(p, (t - LAG) // 2, Bt)
            for pair in range(16 - LAG // 2, 16):
                emit_yd(p, pair, Bt)

            # ---- pass tail; out-DMA split in halves on rotating queues ----
            rcp = outp.tile([128, QP], F32, tag="rcp", name=f"rcp{p}")
            nc.vector.reciprocal_approx_fast(rcp[:], dps[:])
            ynt = outp.tile([128, QP], BF16, tag="ynt", name=f"ynt{p}")
            nc.vector.tensor_tensor(ynt[:], yps[:], rcp[:], ALU.mult)
            for oc in range(2):
                rp = scp.tile([128, QP], F32, tag="sc", name=f"rp{p}_{oc}")
                for ch in range(2):
                    nc.tensor.matmul(
                        rp[:, ts(ch, 512)],
                        wo_sb[:, ts(oc, 128)],
                        ynt[:, ts(ch, 512)],
                        start=True, stop=True)
                ot = outp.tile([128, QP], F32, tag=f"ot{oc}", name=f"ot{p}_{oc}")
                nc.vector.tensor_tensor(
                    ot[:], rp[:], xq_sb[:, oc, ts(p, QP)], ALU.add)
                for ch in range(2):
                    [nc.sync, nc.gpsimd, nc.scalar][(2 * p + oc + ch) % 3].dma_start(
                        out=d["out"][oc][:, p * QP + ch * 512:p * QP + (ch + 1) * 512],
                        in_=ot[:, ts(ch, 512)])


def _prep_in_maps(inputs):
    bf = ml_dtypes.bfloat16
    e4 = ml_dtypes.float8_e4m3
    x = np.ascontiguousarray(np.asarray(inputs["x"], dtype=np.float32))
    w_g = np.asarray(inputs["w_g"], np.float32)
    w_theta = np.asarray(inputs["w_theta"], np.float32)
    w_phi = np.asarray(inputs["w_phi"], np.float32)
    w_out = np.asarray(inputs["w_out"], np.float32)
    for bn in ("b_g", "b_theta", "b_phi", "b_out"):
        assert not np.any(np.asarray(inputs[bn])), f"nonzero {bn} unsupported"

    A = (w_theta.T @ w_phi) * ASC                      # [256, 256]
    # u-projection lhsT: A8[h][kp, j, m] = A[j*128+kp, h*128+m]
    A8 = np.empty((2, 128, 2, 128), e4)
    for h in range(2):
        for j in range(2):
            A8[h, :, j, :] = A[j * 128:(j + 1) * 128,
                               h * 128:(h + 1) * 128].astype(e4)
    wg32 = w_g * GSC                                   # [128, 256]
    wg8 = np.empty((128, 2, 128), e4)
    for kc in range(2):
        wg8[:, kc, :] = wg32[:, kc * 128:(kc + 1) * 128].T.astype(e4)
    wo16 = np.ascontiguousarray((w_out / GSC).T).astype(bf)   # [CI, C]

    in_maps = []
    for c in range(NCORES):
        n, qh = c // 2, c % 2
        xr = x[n].reshape(C, N)
        x8 = xr.astype(e4)
        # key-permute: query window first
        if qh == 1:
            x8 = np.concatenate([x8[:, Q:], x8[:, :Q]], axis=1)
        m = {
            "xb8": np.ascontiguousarray(
                x8.reshape(2, 128, 4, 1024).transpose(0, 2, 1, 3)),
            "xq16": np.ascontiguousarray(
                xr[:, qh * Q:(qh + 1) * Q].astype(bf).reshape(2, 128, Q)),
            "A8": A8, "wg8": wg8, "wo16": wo16,
        }
        in_maps.append(m)
    return in_maps


def _get_nc():
    if "nc" not in _CACHE:
        _CACHE["nc"] = _build()
    return _CACHE["nc"]


def kernel(**inputs):
    in_maps = _prep_in_maps(inputs)
    nc = _get_nc()
    res = run_bass_kernel_spmd(nc, in_maps, list(range(NCORES)))
    out = np.empty((NB, C, N), np.float32)
    for c in range(NCORES):
        n, qh = c // 2, c % 2
        out[n][:, qh * Q:(qh + 1) * Q] = res.results[c]["out"].reshape(C, Q)
    return out.reshape(NB, C, 64, 64)


if __name__ == "__main__":
    rng = np.random.default_rng(0)
    ins = {
        "x": rng.normal(size=(NB, C, 64, 64)).astype(np.float32),
        "w_g": rng.normal(size=(CI, C)).astype(np.float32) * 0.01,
        "b_g": np.zeros(CI, np.float32),
        "w_theta": rng.normal(size=(CI, C)).astype(np.float32) * 0.01,
        "b_theta": np.zeros(CI, np.float32),
        "w_phi": rng.normal(size=(CI, C)).astype(np.float32) * 0.01,
        "b_phi": np.zeros(CI, np.float32),
        "w_out": rng.normal(size=(C, CI)).astype(np.float32) * 0.01,
        "b_out": np.zeros(C, np.float32),
    }
    o = kernel(**ins)
    print("ok", o.shape, o.dtype)


# revision 11
# speedup vs baseline: 3.7456x; 1.0635x over previous
"""NonLocal2D (attention) block on 8 trn2 NeuronCores — fp8 edition.

Sharding: core c -> batch n = c//2, query-half qh = c%2 (2048 of the 4096
spatial positions). Each core gets full x[n] (fp8, key-permuted so the query
window is always sbuf cols 0:2048), plus a bf16 residual slice; produces
out[n][:, qh*2048:(qh+1)*2048] in f32.

Per-core math (weight prep on host):
  A    = (w_theta^T @ w_phi) * 1024            e4m3 [256, 256]
  u    = A^T x[:, qwin]                        [256, Q]  PE DoubleRow -> e4m3
  g    = (w_g*32) @ x                          [128, N]  PE DoubleRow -> e4m3
  s    = x^T u  per s-tile: out[s, q] = x_q^T A x_s = 1024*theta(q).phi(s)
  B    = exp(s*sqrt(128)/1024 - M0)            -> e5m2  (ACT; every 3rd tile
         via DVE int16-Schraudolph + gpsimd clamp -> uint8 bitcast)
  y   += gpair^T B     (DoubleRow e4m3 x e5m2, PSUM accum over 16 pairs)
  d   += ones^T B      (DoubleRow e5m2, PSUM accum)
  ynt  = y * approx(1/d) -> bf16 ; r = (w_out/32)^T ynt ; out = x_bf16 + r

The key (s) order is permuted per-core on host; softmax/y sum over s, so any
order works as long as g-tiles and score-tiles agree.

Queries run in 2 passes of 1024 so PSUM fits:
  scores 2x[128,1024] + y [128,1024] + d [128,1024] = 8 banks = 16KB/part.
"""

import numpy as np
import ml_dtypes

import concourse.bass as bass
import concourse.mybir as mybir
import concourse.tile as tile
from concourse import bacc
from concourse.bass import ts
from concourse.bass_utils import run_bass_kernel_spmd

BF16 = mybir.dt.bfloat16
F32 = mybir.dt.float32
E4 = mybir.dt.float8e4
E5 = mybir.dt.float8e5
I16 = mybir.dt.int16
U8 = mybir.dt.uint8
AF = mybir.ActivationFunctionType
ALU = mybir.AluOpType
DR = mybir.MatmulPerfMode.DoubleRow

C = 256
CI = 128
NB = 4
N = 4096
Q = 2048          # queries per core
QP = 1024         # queries per pass
NCORES = 8

SCALE = float(128 ** 0.5)
ASC = 1024.0      # A prescale
GSC = 32.0        # g prescale
M0 = 15.0         # exp shift: B = exp(s_scaled - M0)
SC_ACT = SCALE / ASC
A8C = 4.0 / float(np.log(2.0))      # e5m2 bits per unit exponent
S_BITS1 = SC_ACT * A8C
S_BITS2 = 60.0 - A8C * M0

_CACHE: dict = {}


def _build():
    nc = bacc.Bacc("TRN2", target_bir_lowering=False, debug=False)

    d = {}
    d["xb8"] = nc.dram_tensor("xb8", [2, 4, 128, 1024], E4,
                              kind="ExternalInput").ap()
    d["xq16"] = nc.dram_tensor("xq16", [2, 128, Q], BF16,
                               kind="ExternalInput").ap()
    d["A8"] = nc.dram_tensor("A8", [2, 128, 2, 128], E4,
                             kind="ExternalInput").ap()
    d["wg8"] = nc.dram_tensor("wg8", [128, 2, 128], E4,
                              kind="ExternalInput").ap()
    d["wo16"] = nc.dram_tensor("wo16", [128, C], BF16,
                               kind="ExternalInput").ap()
    d["out"] = nc.dram_tensor("out", [2, 128, Q], F32, kind="ExternalOutput").ap()

    with tile.TileContext(nc) as tc:
        _body(tc, d)
    nc.compile()
    return nc


def _body(tc, d):
    nc = tc.nc

    with (
        tc.tile_pool(name="const", bufs=1) as const,
        tc.tile_pool(name="acts", bufs=1) as acts,
        tc.tile_pool(name="bp", bufs=1) as bp,
        tc.tile_pool(name="bip", bufs=3) as bip,
        tc.tile_pool(name="outs", bufs=2) as outp,
        tc.tile_pool(name="scp", bufs=2, space="PSUM") as scp,
        tc.tile_pool(name="yd", bufs=1, space="PSUM") as yd,
    ):
        # ---- tiny constants on gpsimd (its queue starts earliest) ----
        wup_l = const.tile([128, 128], BF16, tag="wup_l")
        wup_r = const.tile([128, 256], BF16, tag="wup_r")
        nc.gpsimd.memset(wup_l[:], 1.0)
        nc.gpsimd.memset(wup_r[:], 0.0)
        scratch = const.tile([128, 1], BF16, tag="scratch")
        nc.scalar.activation(scratch[:], wup_l[:, 0:1], AF.Exp, scale=1.0)

        A_sb = const.tile([128, 2, 2, 128], E4, tag="A")     # [kp, h, j, m]
        wg_sb = const.tile([128, 2, 128], E4, tag="wg")
        wo_sb = const.tile([128, C], BF16, tag="wo")
        xb_sb = acts.tile([128, 2, N], E4, tag="xb")         # [kp, kc, s]
        xq_sb = acts.tile([128, 2, Q], BF16, tag="xq")

        # fill: sync gets A8 + kc0 quarters + xq; gpsimd kc1 quarters + w
        nc.sync.dma_start(out=A_sb[:, 0], in_=d["A8"][0])
        nc.sync.dma_start(out=A_sb[:, 1], in_=d["A8"][1])
        nc.gpsimd.dma_start(out=wg_sb[:], in_=d["wg8"][:])
        m0t = const.tile([128, 1], F32, tag="m0")
        ones5 = const.tile([128, 2, 128], E5, tag="ones5")
        nc.gpsimd.memset(m0t[:], -M0)
        for qt in range(4):
            nc.sync.dma_start(out=xb_sb[:, 0, ts(qt, 1024)], in_=d["xb8"][0][qt])
            nc.gpsimd.dma_start(out=xb_sb[:, 1, ts(qt, 1024)], in_=d["xb8"][1][qt])
            if qt == 0:
                nc.gpsimd.memset(ones5[:], 1.0)
            if qt == 1:
                nc.gpsimd.dma_start(out=wo_sb[:], in_=d["wo16"][:])
        for oc in range(2):
            nc.sync.dma_start(out=xq_sb[:, oc], in_=d["xq16"][oc])

        u8_sb = acts.tile([128, 2, Q], E4, tag="u8")         # [kp, h, q]
        g5_sb = acts.tile([128, 16, 2, 128], E4, tag="g5")   # [sp, pair, slot, o]

        yps = yd.tile([128, QP], F32, tag="yps")
        dps = yd.tile([128, QP], F32, tag="dps")

        # ---- PE warm-up during the fill (clock ramp) ----
        for i in range(12):
            nc.tensor.matmul([yps, dps][i % 2][:, 0:256], wup_l[:], wup_r[:],
                             start=True, stop=True)

        def cast(eng, dst, src):
            if eng is nc.scalar:
                nc.scalar.copy(dst, src)
            else:
                eng.tensor_copy(dst, src)

        # ---- u = A^T x[:, 0:2048]: 4 independent psum tiles, casts ACT/DVE ----
        for r in range(2):
            for h in range(2):
                i = 2 * r + h
                pt = ([yps, dps][i] if i < 2 else
                      scp.tile([128, QP], F32, tag="sc", name=f"u{i}"))
                for ch in range(2):
                    nc.tensor.matmul(
                        pt[:, ts(ch, 512)],
                        A_sb[:, h],
                        xb_sb[:, :, r * QP + ch * 512:r * QP + (ch + 1) * 512],
                        start=True, stop=True, perf_mode=DR)
                cast([nc.scalar, nc.vector][h], u8_sb[:, h, ts(r, QP)], pt[:])

        # ---- g: 4 rounds x 8 s-tiles into scp tiles ----
        for b in range(4):
            gp = scp.tile([128, QP], F32, tag="sc", name=f"g{b}")
            for sj in range(8):
                st = b * 8 + sj
                nc.tensor.matmul(
                    gp[:, ts(sj, 128)],
                    xb_sb[:, :, ts(st, 128)],
                    wg_sb[:],
                    start=True, stop=True, perf_mode=DR)
            # s-tiles 8b..8b+7 = pairs 4b..4b+3
            cast([nc.scalar, nc.vector][b % 2],
                 g5_sb[:, 4 * b:4 * b + 4, :, :], gp[:])

        # ---- main: 2 passes x 32 s-tiles.  y/d matmuls are emitted 4 tiles
        # late so PE's in-order queue never parks scores (and thus exp)
        # behind a y/d that waits on B. ----
        def emit_yd(p, pair, Bt):
            for ch in range(2):
                nc.tensor.matmul(
                    yps[:, ts(ch, 512)],
                    g5_sb[:, pair],
                    Bt[pair][:, :, ts(ch, 512)],
                    start=(pair == 0), stop=(pair == 15),
                    perf_mode=DR)
            for ch in range(2):
                nc.tensor.matmul(
                    dps[:, ts(ch, 512)],
                    ones5[:],
                    Bt[pair][:, :, ts(ch, 512)],
                    start=(pair == 0), stop=(pair == 15),
                    perf_mode=DR)

        def emit_proj(p, oc):
            # pass-p projection + residual for one output-channel half;
            # interleaved into the next pass's main loop (rp borrows an scp
            # rotation slot). DMA split per 512 so the drain overlaps.
            ynt = _tails[p]
            rp = scp.tile([128, QP], F32, tag="sc", name=f"rp{p}_{oc}")
            for ch in range(2):
                nc.tensor.matmul(
                    rp[:, ts(ch, 512)],
                    wo_sb[:, ts(oc, 128)],
                    ynt[:, ts(ch, 512)],
                    start=True, stop=True)
            for ch in range(2):
                ot = outp.tile([128, 512], F32, tag=f"ot{oc}{ch}",
                               name=f"ot{p}_{oc}_{ch}")
                nc.vector.tensor_tensor(
                    ot[:], rp[:, ts(ch, 512)],
                    xq_sb[:, oc, p * QP + ch * 512:p * QP + (ch + 1) * 512],
                    ALU.add)
                [nc.sync, nc.gpsimd, nc.scalar][(2 * p + 2 * oc + ch) % 3].dma_start(
                    out=d["out"][oc][:, p * QP + ch * 512:p * QP + (ch + 1) * 512],
                    in_=ot[:])

        _tails = {}

        def emit_ydnorm(p):
            # after the last y/d accumulation: 1/d and y*1/d (DVE)
            rcp = outp.tile([128, QP], F32, tag="rcp", name=f"rcp{p}")
            nc.vector.reciprocal_approx_fast(rcp[:], dps[:])
            ynt = outp.tile([128, QP], BF16, tag="ynt", name=f"ynt{p}")
            nc.vector.tensor_tensor(ynt[:], yps[:], rcp[:], ALU.mult)
            _tails[p] = ynt

        LAG = 4
        for p in range(2):
            qoff = p * QP
            Bt = {}
            for t in range(32):
                pair, slot = t // 2, t % 2
                if slot == 0:
                    Bt[pair] = bp.tile([128, 2, QP], E5, tag=f"B{pair}",
                                       name=f"B{p}_{pair}")
                sc = scp.tile([128, QP], F32, tag="sc", name=f"s{p}_{t}")
                for ch in range(2):
                    nc.tensor.matmul(
                        sc[:, ts(ch, 512)],
                        xb_sb[:, :, ts(t, 128)],
                        u8_sb[:, :, qoff + ch * 512:qoff + (ch + 1) * 512],
                        start=True, stop=True, perf_mode=DR)
                Bslot = Bt[pair][:, slot, :]
                if t % 4 == 2:
                    # offload: DVE Schraudolph bits, then DVE clamp/convert
                    # (gpsimd tensor ops are ~15us each on HW - unusable)
                    bi = bip.tile([128, QP], I16, tag="bi", name=f"bi{p}_{t}")
                    nc.vector.tensor_scalar(
                        out=bi[:], in0=sc[:],
                        scalar1=S_BITS1, scalar2=S_BITS2,
                        op0=ALU.mult, op1=ALU.add)
                    nc.vector.tensor_scalar(
                        out=Bslot.bitcast(U8), in0=bi[:],
                        scalar1=0.0, scalar2=0.0,
                        op0=ALU.max, op1=ALU.add)
                else:
                    nc.scalar.activation(Bslot, sc[:], AF.Exp,
                                         bias=m0t[:], scale=SC_ACT)
                if t >= LAG + 1 and (t - LAG) % 2 == 1:
                    emit_yd(p, (t - LAG) // 2, Bt)
                # pass-1's projection/residual/out-DMA ride inside pass 2
                if p == 1 and t == 6:
                    emit_proj(0, 0)
                if p == 1 and t == 10:
                    emit_proj(0, 1)
            for pair in range(16 - LAG // 2, 16):
                emit_yd(p, pair, Bt)
            emit_ydnorm(p)
        emit_proj(1, 0)
        emit_proj(1, 1)


def _prep_in_maps(inputs):
    bf = ml_dtypes.bfloat16
    e4 = ml_dtypes.float8_e4m3
    x = np.ascontiguousarray(np.asarray(inputs["x"], dtype=np.float32))
    w_g = np.asarray(inputs["w_g"], np.float32)
    w_theta = np.asarray(inputs["w_theta"], np.float32)
    w_phi = np.asarray(inputs["w_phi"], np.float32)
    w_out = np.asarray(inputs["w_out"], np.float32)
    for bn in ("b_g", "b_theta", "b_phi", "b_out"):
        assert not np.any(np.asarray(inputs[bn])), f"nonzero {bn} unsupported"

    A = (w_theta.T @ w_phi) * ASC                      # [256, 256]
    # u-projection lhsT: A8[h][kp, j, m] = A[j*128+kp, h*128+m]
    A8 = np.empty((2, 128, 2, 128), e4)
    for h in range(2):
        for j in range(2):
            A8[h, :, j, :] = A[j * 128:(j + 1) * 128,
                               h * 128:(h + 1) * 128].astype(e4)
    wg32 = w_g * GSC                                   # [128, 256]
    wg8 = np.empty((128, 2, 128), e4)
    for kc in range(2):
        wg8[:, kc, :] = wg32[:, kc * 128:(kc + 1) * 128].T.astype(e4)
    wo16 = np.ascontiguousarray((w_out / GSC).T).astype(bf)   # [CI, C]

    in_maps = []
    for c in range(NCORES):
        n, qh = c // 2, c % 2
        xr = x[n].reshape(C, N)
        x8 = xr.astype(e4)
        # key-permute: query window first
        if qh == 1:
            x8 = np.concatenate([x8[:, Q:], x8[:, :Q]], axis=1)
        m = {
            "xb8": np.ascontiguousarray(
                x8.reshape(2, 128, 4, 1024).transpose(0, 2, 1, 3)),
            "xq16": np.ascontiguousarray(
                xr[:, qh * Q:(qh + 1) * Q].astype(bf).reshape(2, 128, Q)),
            "A8": A8, "wg8": wg8, "wo16": wo16,
        }
        in_maps.append(m)
    return in_maps


def _get_nc():
    if "nc" not in _CACHE:
        _CACHE["nc"] = _build()
    return _CACHE["nc"]


def kernel(**inputs):
    in_maps = _prep_in_maps(inputs)
    nc = _get_nc()
    res = run_bass_kernel_spmd(nc, in_maps, list(range(NCORES)))
    out = np.empty((NB, C, N), np.float32)
    for c in range(NCORES):
        n, qh = c // 2, c % 2
        out[n][:, qh * Q:(qh + 1) * Q] = res.results[c]["out"].reshape(C, Q)
    return out.reshape(NB, C, 64, 64)


if __name__ == "__main__":
    rng = np.random.default_rng(0)
    ins = {
        "x": rng.normal(size=(NB, C, 64, 64)).astype(np.float32),
        "w_g": rng.normal(size=(CI, C)).astype(np.float32) * 0.01,
        "b_g": np.zeros(CI, np.float32),
        "w_theta": rng.normal(size=(CI, C)).astype(np.float32) * 0.01,
        "b_theta": np.zeros(CI, np.float32),
        "w_phi": rng.normal(size=(CI, C)).astype(np.float32) * 0.01,
        "b_phi": np.zeros(CI, np.float32),
        "w_out": rng.normal(size=(C, CI)).astype(np.float32) * 0.01,
        "b_out": np.zeros(C, np.float32),
    }
    o = kernel(**ins)
    print("ok", o.shape, o.dtype)
